# revision 12
# baseline (speedup 1.0000x reference)
"""Trainium2 Bass kernel for nn_DocumentGAT (2-layer multi-head GAT, N=4096,
H=4 heads, D=64, dense random adjacency mask).

Strategy (8 NeuronCores, row-sharded):
  Each core owns 512 destination rows of the N x N attention. Scores are
  computed TRANSPOSED (partition dim = source node j, free dim = dest row i)
  so the att @ Wh contraction needs no transposes at all.

  Key restructurings (validated vs reference to 7e-6 in fp32):
  * exp(leaky_relu(f_s[i]+f_d[j], 0.2)) == max(exp(f_s)exp(f_d),
    exp(.2 f_s)exp(.2 f_d)) exactly (exp monotone + leaky piecewise linear),
    so the N^2 transcendental work collapses to 1-D exps plus cheap DVE
    broadcast-multiplies and a max. A fraction of tiles instead uses the
    direct ACT path (Prelu(alpha=.2) then Exp) to balance engine load.
  * Softmax denominator = an extra ones-column appended to Wh, accumulated by
    the same PE matmul ("65th output").  att-normalization divides by it after
    the matmul (rank-1 reciprocal broadcast via a K=1 matmul).
  * Wh, the ones column, and f_d are produced by ONE augmented matmul per
    j-tile: lhsT = [x^T; ones-row], rhs = [W | ones-ind | W a_dst] per head.
  * elu(x) == max(x, exp(min(x, 0)) - 1) exactly.
  * Layer-0 output is produced directly in h^T layout ([head*64+o, i]), so
    layer-1's Wh matmul consumes it as lhsT with no transpose. The
    inter-layer all-gather of h^T happens on the host (two launches);
    collectives are unavailable in this runtime.

  Mask handling: host pre-converts each core's adj column-slice to a {0,1}
  bf16 [4096, 512] tile; applied as one tensor-tensor multiply on the exp'd
  scores (engine chosen per-tile to balance DVE/GPSIMD).
"""
import numpy as np
import ml_dtypes

import concourse.bass as bass
import concourse.tile as tile
from concourse import bacc, mybir
from concourse.bass_utils import run_bass_kernel_spmd

BF16 = ml_dtypes.bfloat16
F32 = mybir.dt.float32
BF = mybir.dt.bfloat16
AF = mybir.ActivationFunctionType
OP = mybir.AluOpType

N, FIN, H, D = 4096, 20, 4, 64
NC = 8
R = N // NC          # rows per core = 512
JT = N // 128        # j tiles = 32
ALPHA = 0.2

# per-(jt,h) engine assignment, t = jt*4 + h, s = t % 8:
#   s in {0,1,2}: Y-path (ACT Prelu+Exp), mask on GP if s==0 else DVE
#   s in {3..7}:  X-path (DVE ts-mul x2 + tt-max), mask on GP if s in
#                 {3,4,5} else DVE (GPSIMD tensor_tensor: mult/add only)


ASSIGN_MODE = "mix"


def _assign(t):
    if ASSIGN_MODE == "allY":
        return ("Y", None, "dve")
    if ASSIGN_MODE == "allX_nogp":
        return ("X", "dve", "dve")
    if ASSIGN_MODE == "allX":
        return ("X", "dve", "gp" if t % 8 in (0, 3, 4, 5) else "dve")
    s = t % 8
    if s < 3:
        return ("Y", None, "gp" if s == 0 else "dve")
    return ("X", "dve", "gp" if s in (3, 4, 5) else "dve")


def build_layer(layer: int, reps: int = 1, stage: str = "E"):
    """Build the bass program for one GAT layer (SPMD over 8 cores).
    reps > 1 wraps the whole body in a hardware loop (for timing).
    stage: debug gate - truncate after phase B/C/D (E = full)."""
    KA = FIN + 1 if layer == 0 else H * D + 1   # augmented contraction dim
    nc = bacc.Bacc("TRN2", target_bir_lowering=False, debug=False,
                   num_devices=NC)

    maskT = nc.dram_tensor("maskT", [N, R], BF, kind="ExternalInput")
    wA = nc.dram_tensor("wA", [KA, H * 66], F32, kind="ExternalInput")
    wS = nc.dram_tensor("wS", [KA - 1, H], F32, kind="ExternalInput")
    if layer == 0:
        xTa = nc.dram_tensor("xTa", [KA, N], F32, kind="ExternalInput")
        xown = nc.dram_tensor("xown", [KA, R], F32, kind="ExternalInput")
        out = nc.dram_tensor("hout", [H * D, R], F32, kind="ExternalOutput")
    else:
        xTa = nc.dram_tensor("hTa", [KA, N], F32, kind="ExternalInput")
        xown = nc.dram_tensor("hown", [KA - 1, R], F32, kind="ExternalInput")
        out = nc.dram_tensor("oout", [D, R], F32, kind="ExternalOutput")

    with tile.TileContext(nc) as tc:
        with (
            tc.tile_pool(name="res", bufs=1) as res,
            tc.tile_pool(name="work", bufs=4) as work,
            tc.tile_pool(name="glue", bufs=1) as glue,
            tc.tile_pool(name="psA", bufs=4, space="PSUM") as psA,
            tc.tile_pool(name="psB", bufs=2, space="PSUM") as psB,
            tc.tile_pool(name="psC", bufs=2, space="PSUM") as psC,
        ):
            # ---------------- resident tiles -----------------
            mb = res.tile([128, JT * R], BF, tag="mask")          # mask tiles
            wh16 = res.tile([128, JT * 264], BF, tag="wh16")      # [Wh|1|fd] bf16
            fdall = res.tile([128, JT * H], F32, tag="fdall")     # f_d columns
            ball = res.tile([128, JT * H], F32, tag="ball")       # exp(f_d)
            dall = res.tile([128, JT * H], F32, tag="dall")       # exp(.2 f_d)
            a16 = res.tile([128, H * R], BF, tag="a16")           # exp(f_s) bcast
            c16 = res.tile([128, H * R], BF, tag="c16")           # exp(.2 f_s)
            fs32 = res.tile([128, H * R], F32, tag="fs32")        # f_s bcast (Y)
            ones_r = res.tile([1, 128], F32, tag="ones_r")
            ones64 = res.tile([1, 64], F32, tag="ones64")
            if layer == 0:
                xTa_sb = [res.tile([KA, N], F32, tag="xTa", name="xTa_sb")]
                wA_sb = [res.tile([KA, H * 66], F32, tag="wA", name="wA_sb")]
                wS_sb = [res.tile([KA - 1, H], F32, tag="wS", name="wS_sb")]
                xo_sb = [res.tile([KA, R], F32, tag="xo", name="xo_sb")]
            else:
                xTa_sb = [res.tile([128, N], F32, tag="xTa0", name="xTa0"),
                          res.tile([128, N], F32, tag="xTa1", name="xTa1"),
                          res.tile([1, N], F32, tag="xTaO", name="xTaO")]
                wA_sb = [res.tile([128, H * 66], F32, tag="wA0", name="wA0"),
                         res.tile([128, H * 66], F32, tag="wA1", name="wA1"),
                         res.tile([1, H * 66], F32, tag="wAO", name="wAO")]
                wS_sb = [res.tile([128, H], F32, tag="wS0", name="wS0"),
                         res.tile([128, H], F32, tag="wS1", name="wS1")]
                xo_sb = [res.tile([128, R], F32, tag="xo0", name="xo0"),
                         res.tile([128, R], F32, tag="xo1", name="xo1")]
            hout_sb = [res.tile([128, R], F32, tag=f"hout{i}", name=f"hout{i}") for i in range(2)] \
                if layer == 0 else None

            nc.vector.memset(ones_r[:], 1.0)
            nc.vector.memset(ones64[:], 1.0)

            def body():
                # ---------------- phase A: loads -----------------
                for jt in range(JT):
                    nc.sync.dma_start(out=mb[:, jt * R:(jt + 1) * R],
                                      in_=maskT[jt * 128:(jt + 1) * 128, :])
                if layer == 0:
                    nc.sync.dma_start(out=xTa_sb[0][:], in_=xTa[:])
                    nc.sync.dma_start(out=wA_sb[0][:], in_=wA[:])
                    nc.sync.dma_start(out=wS_sb[0][:], in_=wS[:])
                    nc.sync.dma_start(out=xo_sb[0][:], in_=xown[:])
                else:
                    nc.sync.dma_start(out=xTa_sb[0][:], in_=xTa[0:128, :])
                    nc.sync.dma_start(out=xTa_sb[1][:], in_=xTa[128:256, :])
                    nc.sync.dma_start(out=xTa_sb[2][:], in_=xTa[256:257, :])
                    nc.sync.dma_start(out=wA_sb[0][:], in_=wA[0:128, :])
                    nc.sync.dma_start(out=wA_sb[1][:], in_=wA[128:256, :])
                    nc.sync.dma_start(out=wA_sb[2][:], in_=wA[256:257, :])
                    nc.sync.dma_start(out=wS_sb[0][:], in_=wS[0:128, :])
                    nc.sync.dma_start(out=wS_sb[1][:], in_=wS[128:256, :])
                    nc.sync.dma_start(out=xo_sb[0][:], in_=xown[0:128, :])
                    nc.sync.dma_start(out=xo_sb[1][:], in_=xown[128:256, :])

                if stage == "A":
                    nc.vector.memset(hout_sb[0][:] if layer == 0 else None, 0.0)
                    nc.sync.dma_start(out=out[0:128, :], in_=hout_sb[0][:])
                    return
                # -------- phase B: Wh/ones/fd build per j-tile --------
                for jt in range(JT):
                    pw = psB.tile([128, H * 66], F32, tag="whps")
                    nk = len(xTa_sb)
                    for k in range(nk):
                        lhsT = xTa_sb[k][:, jt * 128:(jt + 1) * 128]
                        nc.tensor.matmul(pw[:], lhsT, wA_sb[k][:],
                                         start=(k == 0), stop=(k == nk - 1))
                    nc.vector.tensor_copy(wh16[:, jt * 264:(jt + 1) * 264], pw[:])
                    nc.vector.tensor_copy(fdall[:, jt * H:(jt + 1) * H],
                                          pw[:, 65:65 + 66 * (H - 1) + 1:66])

                if stage == "B":
                    nc.sync.dma_start(out=out[0:128, :] if layer == 0 else out[:, :],
                                      in_=wh16[0:128 if layer == 0 else 64, 0:R])
                    return
                # -------- phase C: 1-D exps + f_s broadcasts --------
                nc.scalar.activation(ball[:], fdall[:], AF.Exp)
                nc.scalar.activation(dall[:], fdall[:], AF.Exp, scale=ALPHA)
                for h in range(H):
                    pf = psC.tile([1, R], F32, tag="psmisc", name="pf")
                    nk = len(wS_sb)
                    for k in range(nk):
                        rhs_fs = xo_sb[k][0:KA - 1, :] if layer == 0 else xo_sb[k][:]
                        nc.tensor.matmul(pf[:], wS_sb[k][:, h:h + 1], rhs_fs,
                                         start=(k == 0), stop=(k == nk - 1))
                    fsrow = glue.tile([1, R], F32, tag="fsrow")
                    nc.vector.tensor_copy(fsrow[:], pf[:])
                    pF = psC.tile([128, R], F32, tag="psmisc", name="pF")
                    nc.tensor.matmul(pF[:], ones_r[:], fsrow[:],
                                     start=True, stop=True)
                    sl = slice(h * R, (h + 1) * R)
                    nc.vector.tensor_copy(fs32[:, sl], pF[:])
                    nc.scalar.activation(a16[:, sl], pF[:], AF.Exp)
                    nc.scalar.activation(c16[:, sl], pF[:], AF.Exp, scale=ALPHA)

                if stage == "C":
                    nc.sync.dma_start(out=out[0:128, :] if layer == 0 else out[:, :],
                                      in_=fs32[0:128 if layer == 0 else 64, 0:R])
                    return
                # -------- phase D: masked softmax numerators + matmul --------
                outp = [psA.tile([65, R], F32, tag="outp", name=f"outp{i}") for i in range(H)]
                for jt in range(JT):
                    msl = mb[:, jt * R:(jt + 1) * R]
                    for h in range(H):
                        t = jt * H + h
                        path, max_eng, mask_eng = _assign(t)
                        hsl = slice(h * R, (h + 1) * R)
                        if path == "Y":
                            ty = work.tile([128, R], F32, tag="ty")
                            nc.scalar.activation(ty[:], fs32[:, hsl], AF.Prelu,
                                                 bias=fdall[:, t:t + 1],
                                                 scale=1.0, alpha=ALPHA)
                            e0 = work.tile([128, R], BF, tag="uy")
                            nc.scalar.activation(e0[:], ty[:], AF.Exp)
                        else:
                            t1 = work.tile([128, R], BF, tag="t1")
                            nc.vector.tensor_scalar(t1[:], a16[:, hsl],
                                                    ball[:, t:t + 1], None, OP.mult)
                            t2 = work.tile([128, R], BF, tag="t2")
                            nc.vector.tensor_scalar(t2[:], c16[:, hsl],
                                                    dall[:, t:t + 1], None, OP.mult)
                            e0 = work.tile([128, R], BF, tag="e0")
                            eng = nc.gpsimd if max_eng == "gp" else nc.vector
                            eng.tensor_tensor(e0[:], t1[:], t2[:], OP.max)
                        em = work.tile([128, R], BF, tag="em")
                        eng = nc.gpsimd if mask_eng == "gp" else nc.vector
                        eng.tensor_tensor(em[:], e0[:], msl, OP.mult)
                        nc.tensor.matmul(outp[h][:],
                                         wh16[:, jt * 264 + h * 66:
                                              jt * 264 + h * 66 + 65],
                                         em[:], start=(jt == 0), stop=(jt == JT - 1))

                if stage == "D":
                    du = glue.tile([65, R], F32, tag="du", name="du")
                    nc.vector.tensor_copy(du[:], outp[0][:])
                    nc.sync.dma_start(out=out[0:65, :] if layer == 0 else out[0:64, :],
                                      in_=du[:] if layer == 0 else du[0:64, :])
                    return
                # -------- phase E: normalize + elu (+avg) + store --------
                elus = []
                for h in range(H):
                    rr = glue.tile([1, R], F32, tag="rr")
                    nc.vector.reciprocal(rr[:], outp[h][64:65, :])
                    pr = psC.tile([64, R], F32, tag="psmisc", name="pr")
                    nc.tensor.matmul(pr[:], ones64[:], rr[:], start=True, stop=True)
                    ou = glue.tile([64, R], F32, tag="ou")
                    nc.vector.tensor_copy(ou[:], outp[h][0:64, :])
                    nm = glue.tile([64, R], F32, tag="nm")
                    nc.vector.tensor_tensor(nm[:], ou[:], pr[:], OP.mult)
                    if stage == "D2":
                        if layer == 0:
                            dst = hout_sb[h // 2][(h % 2) * 64:(h % 2) * 64 + 64, :]
                            nc.vector.tensor_copy(dst, nm[:])
                        continue
                    am = glue.tile([64, R], F32, tag="am")
                    nc.vector.tensor_scalar_min(am[:], nm[:], 0.0)
                    ee = glue.tile([64, R], F32, tag="ee")
                    nc.scalar.activation(ee[:], am[:], AF.Exp)
                    em1 = glue.tile([64, R], F32, tag="em1")
                    nc.vector.tensor_scalar_add(em1[:], ee[:], -1.0)
                    if layer == 0:
                        dst = hout_sb[h // 2][(h % 2) * 64:(h % 2) * 64 + 64, :]
                        nc.vector.tensor_tensor(dst, nm[:], em1[:], OP.max)
                    else:
                        el = glue.tile([64, R], F32, tag=f"elu{h}")
                        nc.vector.tensor_tensor(el[:], nm[:], em1[:], OP.max)
                        elus.append(el)
                if stage == "D2" and layer == 0:
                    nc.sync.dma_start(out=out[0:128, :], in_=hout_sb[0][:])
                    nc.sync.dma_start(out=out[128:256, :], in_=hout_sb[1][:])
                    return
                if layer == 0:
                    nc.sync.dma_start(out=out[0:128, :], in_=hout_sb[0][:])
                    nc.sync.dma_start(out=out[128:256, :], in_=hout_sb[1][:])
                else:
                    s0 = glue.tile([64, R], F32, tag="s0")
                    nc.vector.tensor_tensor(s0[:], elus[0][:], elus[1][:], OP.add)
                    s1 = glue.tile([64, R], F32, tag="s1")
                    nc.vector.tensor_tensor(s1[:], elus[2][:], elus[3][:], OP.add)
                    s2 = glue.tile([64, R], F32, tag="s2")
                    nc.vector.tensor_tensor(s2[:], s0[:], s1[:], OP.add)
                    om = glue.tile([64, R], F32, tag="om")
                    nc.vector.tensor_scalar_mul(om[:], s2[:], 0.25)
                    nc.sync.dma_start(out=out[:], in_=om[:])

            if reps == 1:
                body()
            else:
                with tc.For_i(0, reps, 1):
                    body()
    nc.compile()
    return nc


def _host_prep(x, adj, W0, a0, W1, a1):
    """Precompute augmented weights + per-core mask slices (host, numpy)."""
    x = np.asarray(x, np.float32)
    adj = np.asarray(adj)
    W0 = np.asarray(W0, np.float32)
    a0 = np.asarray(a0, np.float32)
    W1 = np.asarray(W1, np.float32)
    a1 = np.asarray(a1, np.float32)

    masks = [np.ascontiguousarray((adj[c * R:(c + 1) * R, :] > 0).T).astype(BF16)
             for c in range(NC)]

    def aug(W, a, KF):
        # W: [H, KF, 64]; returns wA [KF+1, H*66], wS [KF, H]
        wA = np.zeros((KF + 1, H * 66), np.float32)
        wS = np.zeros((KF, H), np.float32)
        for h in range(H):
            wA[:KF, h * 66:h * 66 + 64] = W[h]
            wA[KF, h * 66 + 64] = 1.0
            wA[:KF, h * 66 + 65] = W[h] @ a[h, D:]
            wS[:, h] = W[h] @ a[h, :D]
        return wA, wS

    wA0, wS0 = aug(W0, a0, FIN)
    wA1, wS1 = aug(W1, a1, H * D)
    xTa = np.concatenate([x.T, np.ones((1, N), np.float32)], 0)
    return masks, xTa, wA0, wS0, wA1, wS1


_cache = {}


def _get(layer):
    if layer not in _cache:
        _cache[layer] = build_layer(layer)
    return _cache[layer]


def kernel(x, adj, W0, a0, W1, a1):
    masks, xTa, wA0, wS0, wA1, wS1 = _host_prep(x, adj, W0, a0, W1, a1)
    cores = list(range(NC))

    # ---- layer 0 ----
    nc0 = _get(0)
    in_maps = [{"maskT": masks[c], "xTa": xTa, "wA": wA0, "wS": wS0,
                "xown": xTa[:, c * R:(c + 1) * R]} for c in cores]
    res0 = run_bass_kernel_spmd(nc0, in_maps, cores).results
    hT = np.concatenate([res0[c]["hout"] for c in cores], axis=1)  # [256, N]

    # ---- layer 1 ----
    nc1 = _get(1)
    hTa = np.concatenate([hT, np.ones((1, N), np.float32)], 0)     # [257, N]
    in_maps = [{"maskT": masks[c], "hTa": hTa, "wA": wA1, "wS": wS1,
                "hown": hT[:, c * R:(c + 1) * R]} for c in cores]
    res1 = run_bass_kernel_spmd(nc1, in_maps, cores).results
    out = np.concatenate([res1[c]["oout"] for c in cores], axis=1)  # [64, N]
    return np.ascontiguousarray(out.T)                              # [N, 64]


# revision 17
# speedup vs baseline: 17.5199x; 17.5199x over previous
"""Trainium2 Bass kernel for nn_DocumentGAT (2-layer multi-head GAT, N=4096,
H=4 heads, D=64, dense random adjacency mask).

Strategy (8 NeuronCores, row-sharded):
  Each core owns 512 destination rows of the N x N attention. Scores are
  computed TRANSPOSED (partition dim = source node j, free dim = dest row i)
  so the att @ Wh contraction needs no transposes at all.

  Key restructurings (validated vs reference to 7e-6 in fp32):
  * exp(leaky_relu(f_s[i]+f_d[j], 0.2)) == max(exp(f_s)exp(f_d),
    exp(.2 f_s)exp(.2 f_d)) exactly (exp monotone + leaky piecewise linear),
    so the N^2 transcendental work collapses to 1-D exps plus cheap DVE
    broadcast-multiplies and a max. A fraction of tiles instead uses the
    direct ACT path (Prelu(alpha=.2) then Exp) to balance engine load.
  * Softmax denominator = an extra ones-column appended to Wh, accumulated by
    the same PE matmul ("65th output").  att-normalization divides by it after
    the matmul (rank-1 reciprocal broadcast via a K=1 matmul).
  * Wh, the ones column, and f_d are produced by ONE augmented matmul per
    j-tile: lhsT = [x^T; ones-row], rhs = [W | ones-ind | W a_dst] per head.
  * elu(x) == max(x, exp(min(x, 0)) - 1) exactly.
  * Layer-0 output is produced directly in h^T layout ([head*64+o, i]), so
    layer-1's Wh matmul consumes it as lhsT with no transpose. The
    inter-layer all-gather of h^T happens on the host (two launches);
    collectives are unavailable in this runtime.

  Mask handling: host pre-converts each core's adj column-slice to a {0,1}
  bf16 [4096, 512] tile; applied as one tensor-tensor multiply on the exp'd
  scores (engine chosen per-tile to balance DVE/GPSIMD).
"""
import numpy as np
import ml_dtypes

import concourse.bass as bass
import concourse.tile as tile
from concourse import bacc, mybir
from concourse.bass_utils import run_bass_kernel_spmd

BF16 = ml_dtypes.bfloat16
F32 = mybir.dt.float32
BF = mybir.dt.bfloat16
AF = mybir.ActivationFunctionType
OP = mybir.AluOpType

N, FIN, H, D = 4096, 20, 4, 64
NC = 8
R = N // NC          # rows per core = 512
JT = N // 128        # j tiles = 32
ALPHA = 0.2

# per-(jt,h) engine assignment, t = jt*4 + h, s = t % 8:
#   s in {0,1,2}: Y-path (ACT Prelu+Exp), mask on GP if s==0 else DVE
#   s in {3..7}:  X-path (DVE ts-mul x2 + tt-max), mask on GP if s in
#                 {3,4,5} else DVE (GPSIMD tensor_tensor: mult/add only)


ASSIGN_MODE = "mix"
N_Y_HEADS = 2     # heads 0..N_Y_HEADS-1 take the ACT (Prelu+Exp) path


def _is_y(h):
    if ASSIGN_MODE == "allY":
        return True
    if ASSIGN_MODE in ("allX", "allX_nogp"):
        return False
    return h < N_Y_HEADS


def build_layer(layer: int, reps: int = 1, stage: str = "E"):
    """Build the bass program for one GAT layer (SPMD over 8 cores).
    reps > 1 wraps the whole body in a hardware loop (for timing).
    stage: debug gate - truncate after phase B/C/D (E = full)."""
    KA = FIN + 1 if layer == 0 else H * D + 1   # augmented contraction dim
    nc = bacc.Bacc("TRN2", target_bir_lowering=False, debug=False,
                   num_devices=NC)

    maskT = nc.dram_tensor("maskT", [N, R], BF, kind="ExternalInput")
    wA = nc.dram_tensor("wA", [KA, H * 66], F32, kind="ExternalInput")
    wS = nc.dram_tensor("wS", [KA - 1, H], F32, kind="ExternalInput")
    if layer == 0:
        xTa = nc.dram_tensor("xTa", [KA, N], F32, kind="ExternalInput")
        xown = nc.dram_tensor("xown", [KA, R], F32, kind="ExternalInput")
        out = nc.dram_tensor("hout", [H * D, R], F32, kind="ExternalOutput")
    else:
        xTa = nc.dram_tensor("hTa", [KA, N], F32, kind="ExternalInput")
        xown = nc.dram_tensor("hown", [KA - 1, R], F32, kind="ExternalInput")
        out = nc.dram_tensor("oout", [D, R], F32, kind="ExternalOutput")

    with tile.TileContext(nc) as tc:
        with (
            tc.tile_pool(name="res", bufs=1) as res,
            tc.tile_pool(name="work", bufs=6) as work,
            tc.tile_pool(name="glue", bufs=1) as glue,
            tc.tile_pool(name="psA", bufs=4, space="PSUM") as psA,
            tc.tile_pool(name="psB", bufs=2, space="PSUM") as psB,
            tc.tile_pool(name="psC", bufs=2, space="PSUM") as psC,
        ):
            # ---------------- resident tiles -----------------
            mb = res.tile([128, JT * R], BF, tag="mask")          # mask tiles
            wh16 = res.tile([128, JT * 264], BF, tag="wh16")      # [Wh|1|fd] bf16
            fdall = res.tile([128, JT * H], F32, tag="fdall")     # f_d columns
            ball = res.tile([128, JT * H], F32, tag="ball")       # exp(f_d)
            dall = res.tile([128, JT * H], F32, tag="dall")       # exp(.2 f_d)
            a16 = res.tile([128, H * R], BF, tag="a16")           # exp(f_s) bcast
            c16 = res.tile([128, H * R], BF, tag="c16")           # exp(.2 f_s)
            fs32 = res.tile([128, H * R], F32, tag="fs32")        # f_s bcast (Y)
            ones_r = res.tile([1, 128], F32, tag="ones_r")
            ones64 = res.tile([1, 64], F32, tag="ones64")
            if layer == 0:
                xTa_sb = [res.tile([KA, N], F32, tag="xTa", name="xTa_sb")]
                wA_sb = [res.tile([KA, H * 66], F32, tag="wA", name="wA_sb")]
                wS_sb = [res.tile([KA - 1, H], F32, tag="wS", name="wS_sb")]
                xo_sb = [res.tile([KA, R], F32, tag="xo", name="xo_sb")]
            else:
                xTa_sb = [res.tile([128, N], F32, tag="xTa0", name="xTa0"),
                          res.tile([128, N], F32, tag="xTa1", name="xTa1"),
                          res.tile([1, N], F32, tag="xTaO", name="xTaO")]
                wA_sb = [res.tile([128, H * 66], F32, tag="wA0", name="wA0"),
                         res.tile([128, H * 66], F32, tag="wA1", name="wA1"),
                         res.tile([1, H * 66], F32, tag="wAO", name="wAO")]
                wS_sb = [res.tile([128, H], F32, tag="wS0", name="wS0"),
                         res.tile([128, H], F32, tag="wS1", name="wS1")]
                xo_sb = [res.tile([128, R], F32, tag="xo0", name="xo0"),
                         res.tile([128, R], F32, tag="xo1", name="xo1")]
            hout_sb = [res.tile([128, R], F32, tag=f"hout{i}", name=f"hout{i}") for i in range(2)] \
                if layer == 0 else None

            nc.vector.memset(ones_r[:], 1.0)
            nc.vector.memset(ones64[:], 1.0)

            def body():
                # ---------------- phase A: loads (weights first) -----------------
                if layer == 0:
                    nc.sync.dma_start(out=xTa_sb[0][:], in_=xTa[:])
                    nc.sync.dma_start(out=wA_sb[0][:], in_=wA[:])
                    nc.sync.dma_start(out=wS_sb[0][:], in_=wS[:])
                    nc.sync.dma_start(out=xo_sb[0][:], in_=xown[:])
                    for jt in range(JT):
                        nc.sync.dma_start(out=mb[:, jt * R:(jt + 1) * R],
                                          in_=maskT[jt * 128:(jt + 1) * 128, :])
                else:
                    nc.sync.dma_start(out=xTa_sb[0][:], in_=xTa[0:128, :])
                    nc.sync.dma_start(out=xTa_sb[1][:], in_=xTa[128:256, :])
                    nc.sync.dma_start(out=xTa_sb[2][:], in_=xTa[256:257, :])
                    nc.sync.dma_start(out=wA_sb[0][:], in_=wA[0:128, :])
                    nc.sync.dma_start(out=wA_sb[1][:], in_=wA[128:256, :])
                    nc.sync.dma_start(out=wA_sb[2][:], in_=wA[256:257, :])
                    nc.sync.dma_start(out=wS_sb[0][:], in_=wS[0:128, :])
                    nc.sync.dma_start(out=wS_sb[1][:], in_=wS[128:256, :])
                    nc.sync.dma_start(out=xo_sb[0][:], in_=xown[0:128, :])
                    nc.sync.dma_start(out=xo_sb[1][:], in_=xown[128:256, :])
                for jt in range(JT):
                    nc.sync.dma_start(out=mb[:, jt * R:(jt + 1) * R],
                                      in_=maskT[jt * 128:(jt + 1) * 128, :])

                if stage == "A":
                    nc.vector.memset(hout_sb[0][:] if layer == 0 else None, 0.0)
                    nc.sync.dma_start(out=out[0:128, :], in_=hout_sb[0][:])
                    return
                # -------- phase C first: f_s rows + broadcasts (indep of B) ----
                if stage == "B":
                    pass
                for h in range(H):
                    pf = psC.tile([1, R], F32, tag="psmisc", name="pf")
                    nk = len(wS_sb)
                    for k in range(nk):
                        rhs_fs = xo_sb[k][0:KA - 1, :] if layer == 0 else xo_sb[k][:]
                        nc.tensor.matmul(pf[:], wS_sb[k][:, h:h + 1], rhs_fs,
                                         start=(k == 0), stop=(k == nk - 1))
                    fsrow = glue.tile([1, R], F32, tag="fsrow")
                    nc.vector.tensor_copy(fsrow[:], pf[:])
                    pF = psC.tile([128, R], F32, tag="psmisc", name="pF")
                    nc.tensor.matmul(pF[:], ones_r[:], fsrow[:],
                                     start=True, stop=True)
                    sl = slice(h * R, (h + 1) * R)
                    nc.vector.tensor_copy(fs32[:, sl], pF[:])
                    nc.scalar.activation(a16[:, sl], pF[:], AF.Exp)
                    nc.scalar.activation(c16[:, sl], pF[:], AF.Exp, scale=ALPHA)

                # -------- phases B+D: per-j-tile pipeline -------
                NY = sum(1 for h in range(H) if _is_y(h))
                outp = [psA.tile([65, R], F32, tag="outp", name=f"outp{i}")
                        for i in range(H)]
                for jt in range(JT):
                    # B: Wh/ones/fd for this j-tile
                    pw = psB.tile([128, H * 66], F32, tag="whps")
                    nk = len(xTa_sb)
                    for k in range(nk):
                        lhsT = xTa_sb[k][:, jt * 128:(jt + 1) * 128]
                        nc.tensor.matmul(pw[:], lhsT, wA_sb[k][:],
                                         start=(k == 0), stop=(k == nk - 1))
                    if jt % 2 == 0:
                        nc.scalar.copy(wh16[:, jt * 264:(jt + 1) * 264], pw[:])
                    else:
                        nc.vector.tensor_copy(
                            wh16[:, jt * 264:(jt + 1) * 264], pw[:])
                    csl = slice(jt * H, (jt + 1) * H)
                    nc.scalar.activation(ball[:, csl],
                                         pw[:, 65:65 + 66 * (H - 1) + 1:66], AF.Exp)
                    nc.scalar.activation(dall[:, csl],
                                         pw[:, 65:65 + 66 * (H - 1) + 1:66], AF.Exp,
                                         scale=ALPHA)
                    if NY:
                        nc.vector.tensor_copy(
                            fdall[:, jt * NY:(jt + 1) * NY],
                            pw[:, 65:65 + 66 * (NY - 1) + 1:66])
                    # D: masked softmax numerators + matmuls
                    e4 = work.tile([128, H * R], BF, tag="e4", name="e4", bufs=3)
                    order = [h for h in range(H) if _is_y(h)] + \
                            [h for h in range(H) if not _is_y(h)]
                    for h in order:
                        t = jt * H + h
                        hsl = slice(h * R, (h + 1) * R)
                        if _is_y(h):
                            ty = work.tile([128, R], F32, tag="ty", name="ty", bufs=4)
                            nc.scalar.activation(ty[:], fs32[:, hsl], AF.Prelu,
                                                 bias=fdall[:, jt * NY + h:
                                                            jt * NY + h + 1],
                                                 scale=1.0, alpha=ALPHA)
                            nc.scalar.activation(e4[:, hsl], ty[:], AF.Exp)
                        else:
                            t1 = work.tile([128, R], BF, tag="t1", name="t1", bufs=4)
                            nc.vector.tensor_scalar(t1[:], a16[:, hsl],
                                                    ball[:, t:t + 1], None,
                                                    OP.mult)
                            nc.vector.scalar_tensor_tensor(
                                e4[:, hsl], c16[:, hsl], dall[:, t:t + 1],
                                t1[:], OP.mult, OP.max)
                    em4 = work.tile([128, H * R], BF, tag="em4", name="em4", bufs=3)
                    msl = mb[:, jt * R:(jt + 1) * R]
                    mrep = bass.AP(tensor=msl.tensor, offset=msl.offset,
                                   ap=[msl.ap[0], [0, H], [1, R]])
                    nc.vector.tensor_tensor(
                        em4.rearrange("p (r f) -> p r f", r=H),
                        e4.rearrange("p (r f) -> p r f", r=H), mrep, OP.mult)
                    for h in range(H):
                        nc.tensor.matmul(outp[h][:],
                                         wh16[:, jt * 264 + h * 66:
                                              jt * 264 + h * 66 + 65],
                                         em4[:, h * R:(h + 1) * R],
                                         start=(jt == 0), stop=(jt == JT - 1))

                # -------- phase E: normalize + elu (+avg) + store --------
                elus = []
                for h in range(H):
                    rr = glue.tile([1, R], F32, tag="rr")
                    nc.vector.reciprocal(rr[:], outp[h][64:65, :])
                    pr = psC.tile([64, R], F32, tag="psmisc", name="pr")
                    nc.tensor.matmul(pr[:], ones64[:], rr[:], start=True, stop=True)
                    ou = glue.tile([64, R], F32, tag="ou")
                    nc.vector.tensor_copy(ou[:], outp[h][0:64, :])
                    nm = glue.tile([64, R], F32, tag="nm")
                    nc.vector.tensor_tensor(nm[:], ou[:], pr[:], OP.mult)
                    if stage == "D2":
                        if layer == 0:
                            dst = hout_sb[h // 2][(h % 2) * 64:(h % 2) * 64 + 64, :]
                            nc.vector.tensor_copy(dst, nm[:])
                        continue
                    am = glue.tile([64, R], F32, tag="am")
                    nc.vector.tensor_scalar_min(am[:], nm[:], 0.0)
                    ee = glue.tile([64, R], F32, tag="ee")
                    nc.scalar.activation(ee[:], am[:], AF.Exp)
                    em1 = glue.tile([64, R], F32, tag="em1")
                    nc.vector.tensor_scalar_add(em1[:], ee[:], -1.0)
                    if layer == 0:
                        dst = hout_sb[h // 2][(h % 2) * 64:(h % 2) * 64 + 64, :]
                        nc.vector.tensor_tensor(dst, nm[:], em1[:], OP.max)
                    else:
                        el = glue.tile([64, R], F32, tag=f"elu{h}")
                        nc.vector.tensor_tensor(el[:], nm[:], em1[:], OP.max)
                        elus.append(el)
                if stage == "D2" and layer == 0:
                    nc.sync.dma_start(out=out[0:128, :], in_=hout_sb[0][:])
                    nc.sync.dma_start(out=out[128:256, :], in_=hout_sb[1][:])
                    return
                if layer == 0:
                    nc.sync.dma_start(out=out[0:128, :], in_=hout_sb[0][:])
                    nc.sync.dma_start(out=out[128:256, :], in_=hout_sb[1][:])
                else:
                    s0 = glue.tile([64, R], F32, tag="s0")
                    nc.vector.tensor_tensor(s0[:], elus[0][:], elus[1][:], OP.add)
                    s1 = glue.tile([64, R], F32, tag="s1")
                    nc.vector.tensor_tensor(s1[:], elus[2][:], elus[3][:], OP.add)
                    s2 = glue.tile([64, R], F32, tag="s2")
                    nc.vector.tensor_tensor(s2[:], s0[:], s1[:], OP.add)
                    om = glue.tile([64, R], F32, tag="om")
                    nc.vector.tensor_scalar_mul(om[:], s2[:], 0.25)
                    nc.sync.dma_start(out=out[:], in_=om[:])

            if reps == 1:
                body()
            else:
                with tc.For_i(0, reps, 1):
                    body()
    nc.compile()
    return nc


def _host_prep(x, adj, W0, a0, W1, a1):
    """Precompute augmented weights + per-core mask slices (host, numpy)."""
    x = np.asarray(x, np.float32)
    adj = np.asarray(adj)
    W0 = np.asarray(W0, np.float32)
    a0 = np.asarray(a0, np.float32)
    W1 = np.asarray(W1, np.float32)
    a1 = np.asarray(a1, np.float32)

    masks = [np.ascontiguousarray((adj[c * R:(c + 1) * R, :] > 0).T).astype(BF16)
             for c in range(NC)]

    def aug(W, a, KF):
        # W: [H, KF, 64]; returns wA [KF+1, H*66], wS [KF, H]
        wA = np.zeros((KF + 1, H * 66), np.float32)
        wS = np.zeros((KF, H), np.float32)
        for h in range(H):
            wA[:KF, h * 66:h * 66 + 64] = W[h]
            wA[KF, h * 66 + 64] = 1.0
            wA[:KF, h * 66 + 65] = W[h] @ a[h, D:]
            wS[:, h] = W[h] @ a[h, :D]
        return wA, wS

    wA0, wS0 = aug(W0, a0, FIN)
    wA1, wS1 = aug(W1, a1, H * D)
    xTa = np.concatenate([x.T, np.ones((1, N), np.float32)], 0)
    return masks, xTa, wA0, wS0, wA1, wS1


_cache = {}


def _get(layer):
    if layer not in _cache:
        _cache[layer] = build_layer(layer)
    return _cache[layer]


def kernel(x, adj, W0, a0, W1, a1):
    masks, xTa, wA0, wS0, wA1, wS1 = _host_prep(x, adj, W0, a0, W1, a1)
    cores = list(range(NC))

    # ---- layer 0 ----
    nc0 = _get(0)
    in_maps = [{"maskT": masks[c], "xTa": xTa, "wA": wA0, "wS": wS0,
                "xown": xTa[:, c * R:(c + 1) * R]} for c in cores]
    res0 = run_bass_kernel_spmd(nc0, in_maps, cores).results
    hT = np.concatenate([res0[c]["hout"] for c in cores], axis=1)  # [256, N]

    # ---- layer 1 ----
    nc1 = _get(1)
    hTa = np.concatenate([hT, np.ones((1, N), np.float32)], 0)     # [257, N]
    in_maps = [{"maskT": masks[c], "hTa": hTa, "wA": wA1, "wS": wS1,
                "hown": hT[:, c * R:(c + 1) * R]} for c in cores]
    res1 = run_bass_kernel_spmd(nc1, in_maps, cores).results
    out = np.concatenate([res1[c]["oout"] for c in cores], axis=1)  # [64, N]
    return np.ascontiguousarray(out.T)                              # [N, 64]
